# revision 26
# baseline (speedup 1.0000x reference)
"""Bass/Trainium2 kernel for nn_DistanceTransformerEncoderBlock.

Model (see reference): two neighbour-attention blocks + MLPs over a chain
graph: forward block maps 2048 node sources -> 64512 pair targets (K=16
neighbours), reverse block maps pair sources -> 2048 node targets.

Sharding: pair rows and node rows are sharded contiguously (by segment)
across the 8 cores; the small node k/v table and all weights are
replicated.  The dominant cost is the neighbour gather
(64512*16 k/v rows); it is done on-device with `dma_gather` from a
device-built bf16-packed [k|v] table in DRAM.

Layout trick: gather indices are ordered so that gathered row
g = (t_hi*16 + j)*128 + t_lo lands at partition t_lo, free slot
(t_hi*16 + j).  With attention features permuted to (d-major, head-minor)
column order, every attention reduction (over head-dim d and over
neighbours j) is a free-dim segment reduction done as a tensor-tensor
add tree on the vector engine; no cross-partition ops are needed.
"""

import functools
import os
import sys

sys.path.insert(0, "/opt/trn_rl_repo")

import ml_dtypes
import numpy as np

# ---------------------------------------------------------------- constants
N_SEG = 32
L = 64
N = N_SEG * L                     # 2048 nodes
PPS = L * (L - 1) // 2            # 2016 pairs per segment
P = N_SEG * PPS                   # 64512 pair rows
K = 16                            # neighbours
SIZE = 128
DSIZE = 128
HEADS = 8
HDIM = 16
ATTN = 128
HIDDEN = 128

NCORES = 8
PD = P // NCORES                  # 8064 pair rows per core
ND = N // NCORES                  # 256 node rows per core

FW_TILES = PD // 128              # 63 target tiles
FW_TPC = 4                        # tiles per chunk
FW_CHUNKS = FW_TILES // FW_TPC    # 15 full chunks... 63 = 15*4+3
RV_TILES = ND // 128              # 2

BF16 = ml_dtypes.bfloat16

# feature permutation: position i = d*8 + h holds original feature h*16 + d
_i = np.arange(ATTN)
PERM = ((_i % HEADS) * HDIM + (_i // HEADS)).astype(np.int64)  # old index


def _f32(x):
    return np.ascontiguousarray(np.asarray(x, dtype=np.float32))


def _bf(x):
    return np.ascontiguousarray(np.asarray(x, dtype=np.float32).astype(BF16))


@functools.lru_cache(maxsize=1)
def _noise():
    import jax
    import jax.numpy as jnp

    cpu = jax.local_devices(backend="cpu")[0]
    with jax.default_device(cpu):
        nd = 0.1 * jax.random.normal(jax.random.key(1), (P, DSIZE), jnp.float32)
        nn_ = 0.1 * jax.random.normal(jax.random.key(2), (N, SIZE), jnp.float32)
        return np.asarray(nd), np.asarray(nn_)


@functools.lru_cache(maxsize=1)
def _pair_idx():
    a, b = np.triu_indices(L, k=1)
    off = (np.arange(N_SEG) * L)[:, None]
    idx_i = (off + a[None, :]).reshape(-1)
    idx_j = (off + b[None, :]).reshape(-1)
    return idx_i, idx_j


def _wrap_idx16(flat):
    """flat [G] -> int16 [128, G//16]; [p, s] = flat[s*16 + p%16].

    dma_gather reads a [128, G/16] index AP: the wrapped 16-partition
    block, replicated 8x across the 128 partitions.
    """
    g = flat.shape[0]
    w = flat.reshape(g // 16, 16).T.astype(np.int16)
    return np.ascontiguousarray(np.tile(w, (8, 1)))


def _gather_order(struct):
    """struct [T, K] -> flat gather list with g=(t_hi*K+j)*128+t_lo."""
    t = struct.shape[0]
    return np.ascontiguousarray(
        struct.reshape(t // 128, 128, K).transpose(0, 2, 1)
    ).reshape(-1)


_STAGE = int(os.environ.get("KSTAGE", "9"))


# ================================================================ program
@functools.lru_cache(maxsize=1)
def _build_program():
    import concourse.bacc as bacc
    import concourse.mybir as mybir
    import concourse.tile as tile
    from concourse.masks import make_identity

    dt = mybir.dt
    Alu = mybir.AluOpType
    Act = mybir.ActivationFunctionType

    nc = bacc.Bacc("TRN2", target_bir_lowering=False, debug=False,
                   num_devices=NCORES)

    def din(name, shape, dty):
        return nc.dram_tensor(name, list(shape), dty, kind="ExternalInput").ap()

    def dout(name, shape, dty):
        return nc.dram_tensor(name, list(shape), dty, kind="ExternalOutput").ap()

    # ------------- inputs (per core)
    dfT = din("dfT", [128, PD], dt.float32)
    noiseT = din("noiseT", [128, PD], dt.float32)
    nfT16 = din("nfT16", [128, N], dt.bfloat16)
    posones = din("posones", [2, N], dt.bfloat16)
    nfTs = din("nfTs", [128, ND], dt.float32)
    noisenT = din("noisenT", [128, ND], dt.float32)
    fwidx = din("fwidx", [128, PD * K // 16], dt.int16)
    rvidx = din("rvidx", [128, ND * K // 16], dt.int16)
    rawuT16 = din("rawuT16", [128, ND * K], dt.bfloat16)
    pos3u = din("pos3u", [3, ND * K], dt.bfloat16)

    # all small weights packed in one bf16 blob, all biases in one f32 blob
    # blob16 column map (each 128 wide unless noted):
    # 0: wq_f | 128: bqrep_f | 256: wkv_f(256) | 512: wkvp_f rows0-1(256)
    # 768: wo_f | 896: w1_f | 1024: w2_f | 1152: w3_f
    # 1280: wq_r | 1408: bqrep_r | 1536: wkv_r(256) | 1792: wkvp_r rows0-2(256)
    # 2048: wo_r | 2176: w1_r | 2304: w2_r | 2432: w3_r    -> total 2560
    blob16 = din("blob16", [128, 2560], dt.bfloat16)
    blob32 = din("blob32", [128, 8], dt.float32)

    out_d = dout("out_d", [PD, DSIZE], dt.float32)
    out_n = dout("out_n", [ND, SIZE], dt.float32)

    with tile.TileContext(nc) as tc:
        cst = tc.alloc_tile_pool(name="cst", bufs=1)
        dram = tc.alloc_tile_pool(name="dram", bufs=1, space="DRAM")
        psq = tc.alloc_tile_pool(name="psq", bufs=1, space="PSUM")    # [128,512]
        psmm = tc.alloc_tile_pool(name="psmm", bufs=2, space="PSUM")  # [128,512]
        pskv = tc.alloc_tile_pool(name="pskv", bufs=1, space="PSUM")  # [128,256]
        pstp = tc.alloc_tile_pool(name="pstp", bufs=2, space="PSUM")  # [128,128]
        sb = tc.alloc_tile_pool(name="sb", bufs=2)
        sbig = tc.alloc_tile_pool(name="sbig", bufs=2)

        def sbuf_const(ap, dty=None, pool=cst):
            t = pool.tile(list(ap.shape), dty or ap.dtype, tag=ap.name)
            nc.sync.dma_start(t[...], ap[...])
            return t

        # weights / biases in SBUF via two blob DMAs
        W = {}
        for ap in (nfT16, posones, rawuT16, pos3u):
            W[ap.name] = sbuf_const(ap)
        b16 = cst.tile([128, 2560], dt.bfloat16, tag="blob16")
        nc.sync.dma_start(b16[...], blob16[...])
        b32 = cst.tile([128, 8], dt.float32, tag="blob32")
        nc.sync.dma_start(b32[...], blob32[...])
        _c = [0]

        def _blob(width, rows=128):
            off = _c[0]
            _c[0] += width
            return b16[:rows, off:off + width]

        for pre in ("f", "r"):
            W["wq_" + pre] = _blob(128)
            W["bqrep_" + pre] = _blob(128)
            W["wkv_" + pre] = _blob(256)
            W["wkvp_" + pre] = _blob(256, rows=2 if pre == "f" else 3)
            W["wo_" + pre] = _blob(128)
            W["w1_" + pre] = _blob(128)
            W["w2_" + pre] = _blob(128)
            W["w3_" + pre] = _blob(128)
        for i, nm in enumerate(("bo_f", "b1_f", "b2_f", "b3_f",
                                "bo_r", "b1_r", "b2_r", "b3_r")):
            W[nm] = b32[:, i:i + 1]

        ident16 = cst.tile([128, 128], dt.bfloat16, tag="id16")
        make_identity(nc, ident16[...])
        ident32 = cst.tile([128, 128], dt.float32, tag="id32")
        make_identity(nc, ident32[...])

        # ---------------- k|v source tables -> DRAM (bf16 packed 256 cols)
        kvn_dram = dram.tile([N, 256], dt.bfloat16, tag="kvn")
        kvu_dram = dram.tile([ND * K, 256], dt.bfloat16, tag="kvu")

        def build_table(n_rows, featT, pos_rows, wkv, wkvp, dst):
            for i4 in range(n_rows // 512):
                rows = sb.tile([128, 4, 256], dt.bfloat16, tag="kvrow", bufs=2)
                for k4 in range(4):
                    i = i4 * 4 + k4
                    ps = pskv.tile([128, 256], dt.float32, tag="pskv")
                    nc.tensor.matmul(ps[...],
                                     lhsT=featT[:, i * 128:(i + 1) * 128],
                                     rhs=wkv, start=True, stop=False)
                    nc.tensor.matmul(ps[...],
                                     lhsT=pos_rows[:, i * 128:(i + 1) * 128],
                                     rhs=wkvp, start=False, stop=True)
                    nc.scalar.activation(rows[:, k4, :], ps[...], Act.Copy)
                nc.sync.dma_start(
                    dst[i4 * 512:(i4 + 1) * 512, :].rearrange(
                        "(k p) e -> p k e", p=128),
                    rows[...])

        build_table(N, W["nfT16"][...], W["posones"][...], W["wkv_f"], W["wkvp_f"],
                    kvn_dram)
        build_table(ND * K, W["rawuT16"][...], W["pos3u"][...], W["wkv_r"], W["wkvp_r"],
                    kvu_dram)

        # ---------------- one attention+mlp block over a chunk of targets
        def block_chunk(cfg, c, tpc):
            """Process tiles [c*FW_TPC, c*FW_TPC+tpc) of a block."""
            t0 = c * FW_TPC * 128          # first target of chunk
            nt = tpc * 128                 # targets in chunk
            gpt = 128 * K                  # gathered rows per tile

            # residual input chunk: xT = x_inT + noiseT  (f32, transposed)
            xT = sb.tile([128, nt], dt.float32, tag="xT", bufs=2)
            nz = sb.tile([128, nt], dt.float32, tag="nz", bufs=2)
            nc.sync.dma_start(xT[...], cfg["xT"][:, t0:t0 + nt])
            nc.sync.dma_start(nz[...], cfg["noiseT"][:, t0:t0 + nt])
            nc.vector.tensor_add(xT[...], xT[...], nz[...])
            xT16 = sb.tile([128, nt], dt.bfloat16, tag="xT16", bufs=2)
            nc.scalar.activation(xT16[...], xT[...], Act.Copy)
            if _STAGE < 2:
                return

            # q rows (t on partition), scaled by 1/sqrt(HDIM) via weights
            psq_t = psq.tile([128, nt], dt.float32, tag="psq")
            for i in range(tpc):
                nc.tensor.matmul(psq_t[:, i * 128:(i + 1) * 128],
                                 lhsT=xT16[:, i * 128:(i + 1) * 128],
                                 rhs=cfg["wq"], start=True, stop=True)
            q16 = sb.tile([128, nt], dt.bfloat16, tag="q16", bufs=2)
            nc.vector.tensor_tensor(
                q16[...].rearrange("p (a e) -> p a e", e=128),
                psq_t[...].rearrange("p (a e) -> p a e", e=128),
                cfg["bqrep"].unsqueeze(1).broadcast_to([128, tpc, 128]),
                op=Alu.add)

            if _STAGE < 3:
                return
            # neighbour k|v gather for this chunk
            idxs = cfg["idxs"]
            kv = sbig.tile([128, tpc * K, 256], dt.bfloat16, tag="kv", bufs=2)
            nc.gpsimd.dma_gather(kv[...],
                                 cfg["table"][...],
                                 idxs[:, t0 * K // 16:(t0 + nt) * K // 16],
                                 num_idxs=nt * K, num_idxs_reg=nt * K,
                                 elem_size=256, single_packet=False)

            if _STAGE < 4:
                return
            kvr = kv[...].rearrange("p (a j) e -> p a j e", j=K)
            # prod_k = kg * q   [128, tpc, K, 128]
            prod = sbig.tile([128, tpc * K * 128], dt.bfloat16, tag="big",
                             bufs=3)
            prod_v = prod[...].rearrange("p (a j e) -> p a j e", j=K, e=128)
            nc.vector.tensor_tensor(
                prod_v, kvr[:, :, :, 0:128],
                q16[...].rearrange("p (a e) -> p a e", e=128)
                   .unsqueeze(2).broadcast_to([128, tpc, K, 128]),
                op=Alu.mult)

            # logits = segment-sum over d (stride HEADS): in-place add tree
            p5 = prod[...].rearrange("p (a j d h) -> p a j d h", j=K, d=HDIM,
                                     h=HEADS)
            nc.vector.tensor_add(p5[:, :, :, 0:8, :], p5[:, :, :, 0:8, :],
                                 p5[:, :, :, 8:16, :])
            nc.vector.tensor_add(p5[:, :, :, 0:4, :], p5[:, :, :, 0:4, :],
                                 p5[:, :, :, 4:8, :])
            nc.vector.tensor_add(p5[:, :, :, 0:2, :], p5[:, :, :, 0:2, :],
                                 p5[:, :, :, 2:4, :])
            logit = sb.tile([128, tpc * K * HEADS], dt.float32, tag="logit",
                            bufs=2)
            lg4 = logit[...].rearrange("p (a j h) -> p a j h", j=K, h=HEADS)
            nc.vector.tensor_add(lg4, p5[:, :, :, 0, :], p5[:, :, :, 1, :])

            if _STAGE < 5:
                return
            # softmax over j (no max subtraction; logits are O(1))
            expv = sb.tile([128, tpc * K * HEADS], dt.bfloat16, tag="expv",
                           bufs=2)
            nc.scalar.activation(expv[...], logit[...], Act.Exp)
            e4 = expv[...].rearrange("p (a j h) -> p a j h", j=K, h=HEADS)
            dA = sb.tile([128, tpc * 8 * HEADS], dt.bfloat16, tag="dA", bufs=2)
            dA4 = dA[...].rearrange("p (a j h) -> p a j h", j=8, h=HEADS)
            nc.vector.tensor_add(dA4, e4[:, :, 0:8, :], e4[:, :, 8:16, :])
            dB = sb.tile([128, tpc * 4 * HEADS], dt.bfloat16, tag="dB", bufs=2)
            dB4 = dB[...].rearrange("p (a j h) -> p a j h", j=4, h=HEADS)
            nc.vector.tensor_add(dB4, dA4[:, :, 0:4, :], dA4[:, :, 4:8, :])
            den = sb.tile([128, tpc * HEADS], dt.float32, tag="den", bufs=2)
            dn3 = den[...].rearrange("p (a h) -> p a h", h=HEADS)
            # last two levels fused: (j0+j1) + (j2+j3) in f32
            dC = sb.tile([128, tpc * 2 * HEADS], dt.float32, tag="dC", bufs=2)
            dC4 = dC[...].rearrange("p (a j h) -> p a j h", j=2, h=HEADS)
            nc.vector.tensor_add(dC4, dB4[:, :, 0:2, :], dB4[:, :, 2:4, :])
            nc.vector.tensor_add(dn3, dC4[:, :, 0, :], dC4[:, :, 1, :])
            rden = sb.tile([128, tpc * HEADS], dt.float32, tag="rden", bufs=2)
            nc.vector.reciprocal(rden[...], den[...])
            rden16 = sb.tile([128, tpc * HEADS], dt.bfloat16, tag="rden16",
                             bufs=2)
            nc.vector.tensor_copy(rden16[...], rden[...])

            wgt = sb.tile([128, tpc * K * HEADS], dt.bfloat16, tag="wgt",
                          bufs=2)
            w4 = wgt[...].rearrange("p (a j h) -> p a j h", j=K, h=HEADS)
            nc.vector.tensor_tensor(
                w4, e4,
                rden16[...].rearrange("p (a h) -> p a h", h=HEADS)
                    .unsqueeze(2).broadcast_to([128, tpc, K, HEADS]),
                op=Alu.mult)

            if _STAGE < 6:
                return
            # prod_v = vg * w ; sum over j -> o [128, tpc, 128]
            pv = sbig.tile([128, tpc * K * 128], dt.bfloat16, tag="big",
                           bufs=3)
            pv5 = pv[...].rearrange("p (a j d h) -> p a j d h", j=K, d=HDIM,
                                    h=HEADS)
            nc.vector.tensor_tensor(
                pv5,
                kvr[:, :, :, 128:256].rearrange("p a j (d h) -> p a j d h",
                                                h=HEADS),
                w4.unsqueeze(3).broadcast_to([128, tpc, K, HDIM, HEADS]),
                op=Alu.mult)
            pv4 = pv[...].rearrange("p (a j e) -> p a j e", j=K, e=128)
            nc.vector.tensor_add(pv4[:, :, 0:8, :], pv4[:, :, 0:8, :],
                                 pv4[:, :, 8:16, :])
            nc.vector.tensor_add(pv4[:, :, 0:4, :], pv4[:, :, 0:4, :],
                                 pv4[:, :, 4:8, :])
            nc.vector.tensor_add(pv4[:, :, 0:2, :], pv4[:, :, 0:2, :],
                                 pv4[:, :, 2:4, :])
            o16 = sb.tile([128, tpc * 128], dt.bfloat16, tag="o16", bufs=2)
            o3 = o16[...].rearrange("p (a e) -> p a e", e=128)
            nc.vector.tensor_add(o3, pv4[:, :, 0, :], pv4[:, :, 1, :])

            if _STAGE < 7:
                return
            # transpose o tiles -> oT [128(dh), nt]
            oT = sb.tile([128, nt], dt.bfloat16, tag="oT", bufs=2)
            for i in range(tpc):
                pst = pstp.tile([128, 128], dt.bfloat16, tag="pstp16")
                nc.tensor.transpose(pst[...], o16[...][:, i * 128:(i + 1) * 128],
                                    ident16[...])
                nc.scalar.activation(oT[...][:, i * 128:(i + 1) * 128],
                                     pst[...], Act.Copy)

            # attn_T = wo^T-contract -> [e, t]; residual x0T = attn+bo+xT
            psa = psmm.tile([128, nt], dt.float32, tag="psmm")
            nc.tensor.matmul(psa[...], lhsT=cfg["wo"], rhs=oT[...],
                             start=True, stop=True)
            x0T = sb.tile([128, nt], dt.float32, tag="x0T", bufs=2)
            nc.vector.scalar_tensor_tensor(x0T[...], psa[...], cfg["bo"],
                                           xT[...], op0=Alu.add, op1=Alu.add)
            x0T16 = sb.tile([128, nt], dt.bfloat16, tag="x0T16", bufs=2)
            nc.scalar.activation(x0T16[...], x0T[...], Act.Copy)

            if _STAGE < 8:
                return
            # MLP (transposed): h = relu(W x + b)
            ps1 = psmm.tile([128, nt], dt.float32, tag="psmm")
            nc.tensor.matmul(ps1[...], lhsT=cfg["w1"], rhs=x0T16[...],
                             start=True, stop=True)
            h1 = sb.tile([128, nt], dt.bfloat16, tag="h1", bufs=2)
            nc.scalar.activation(h1[...], ps1[...], Act.Relu, bias=cfg["b1"])
            ps2 = psmm.tile([128, nt], dt.float32, tag="psmm")
            nc.tensor.matmul(ps2[...], lhsT=cfg["w2"], rhs=h1[...],
                             start=True, stop=True)
            h2 = sb.tile([128, nt], dt.bfloat16, tag="h2", bufs=2)
            nc.scalar.activation(h2[...], ps2[...], Act.Relu, bias=cfg["b2"])
            ps3 = psmm.tile([128, nt], dt.float32, tag="psmm")
            nc.tensor.matmul(ps3[...], lhsT=cfg["w3"], rhs=h2[...],
                             start=True, stop=True)
            yT = sb.tile([128, nt], dt.float32, tag="yT", bufs=2)
            nc.vector.scalar_tensor_tensor(yT[...], ps3[...], cfg["b3"],
                                           x0T[...], op0=Alu.add, op1=Alu.add)

            if _STAGE < 9:
                return
            # transpose back to rows and store
            yrows = sb.tile([128, tpc * 128], dt.float32, tag="yrows", bufs=2)
            yr3 = yrows[...].rearrange("p (a e) -> p a e", e=128)
            for i in range(tpc):
                pst = pstp.tile([128, 128], dt.float32, tag="pstp")
                nc.tensor.transpose(pst[...], yT[...][:, i * 128:(i + 1) * 128],
                                    ident32[...])
                if i % 2 == 0:
                    nc.scalar.activation(yr3[:, i, :], pst[...], Act.Copy)
                else:
                    nc.vector.tensor_copy(yr3[:, i, :], pst[...])
            nc.sync.dma_start(
                cfg["out"].rearrange("(a p) e -> p a e", p=128)
                          [:, c * FW_TPC:c * FW_TPC + tpc, :],
                yr3)

        fwidx_sb = cst.tile([128, PD * K // 16], dt.int16, tag="fwidx")
        nc.sync.dma_start(fwidx_sb[...], fwidx[...])
        rvidx_sb = cst.tile([128, ND * K // 16], dt.int16, tag="rvidx")
        nc.sync.dma_start(rvidx_sb[...], rvidx[...])
        fw_cfg = dict(xT=dfT, noiseT=noiseT, idxs=fwidx_sb[...], table=kvn_dram,
                      wq=W["wq_f"], bqrep=W["bqrep_f"], wo=W["wo_f"],
                      bo=W["bo_f"], w1=W["w1_f"], b1=W["b1_f"], w2=W["w2_f"],
                      b2=W["b2_f"], w3=W["w3_f"], b3=W["b3_f"], out=out_d)
        rv_cfg = dict(xT=nfTs, noiseT=noisenT, idxs=rvidx_sb[...], table=kvu_dram,
                      wq=W["wq_r"], bqrep=W["bqrep_r"], wo=W["wo_r"],
                      bo=W["bo_r"], w1=W["w1_r"], b1=W["b1_r"], w2=W["w2_r"],
                      b2=W["b2_r"], w3=W["w3_r"], b3=W["b3_r"], out=out_n)

        full, rem = divmod(FW_TILES, FW_TPC)
        for _rep in range(int(os.environ.get("KREPS", "1"))):
            for c in range(full):
                block_chunk(fw_cfg, c, FW_TPC)
            if rem:
                block_chunk(fw_cfg, full, rem)
            block_chunk(rv_cfg, 0, RV_TILES)

        for pool in (sbig, sb, pstp, pskv, psmm, psq, dram, cst):
            pool.release()

    nc.compile()
    return nc


# ============================================================= host wrapper
def _prepare(inputs):
    node_features = _f32(inputs["node_features"])
    distance_features = _f32(inputs["distance_features"])
    node_structure = np.asarray(inputs["node_structure"])
    distance_structure = np.asarray(inputs["distance_structure"])
    seg = np.asarray(inputs["subgraph_indices"]).astype(np.int64)
    fw = inputs["fw_params"]
    rv = inputs["rv_params"]

    noise_d, noise_n = _noise()
    idx_i, idx_j = _pair_idx()

    # irange / positions (matches reference.irange for sorted contiguous seg)
    counts = np.bincount(seg, minlength=N_SEG)
    shift = np.cumsum(counts) - counts
    node_pos = (np.arange(N) - shift[seg]).astype(np.float32) / 64.0
    p0 = node_pos[idx_j] / 64.0
    p1 = node_pos[idx_i] / 64.0

    def attn_w(p, scale_q):
        wq = _f32(p["wq"])[:, PERM]
        bq = _f32(p["bq"])[PERM]
        if scale_q:
            wq = wq * 0.25
            bq = bq * 0.25
        wk = _f32(p["wk"])[:, PERM]
        wv = _f32(p["wv"])[:, PERM]
        bk = _f32(p["bk"])[PERM]
        bv = _f32(p["bv"])[PERM]
        wo = _f32(p["wo"])[PERM, :]
        bo = _f32(p["bo"])
        return wq, bq, wk, wv, bk, bv, wo, bo

    wq_f, bq_f, wk_f, wv_f, bk_f, bv_f, wo_f, bo_f = attn_w(fw["attn"], True)
    wq_r, bq_r, wk_r, wv_r, bk_r, bv_r, wo_r, bo_r = attn_w(rv["attn"], True)

    # fw k/v sources: node_pos_features = [node_features, node_pos] (129 dims)
    wkv_f = _bf(np.concatenate([wk_f[:SIZE], wv_f[:SIZE]], axis=1))
    wkvp_f = _bf(np.stack([np.concatenate([wk_f[SIZE], wv_f[SIZE]]),
                           np.concatenate([bk_f, bv_f])]))
    # rv k/v sources: dist_pos_features = [distance_features, p0, p1] (130)
    wkv_r_c = _bf(np.concatenate([wk_r[:DSIZE], wv_r[:DSIZE]], axis=1))
    wkvp_r = _bf(np.stack([
        np.concatenate([wk_r[DSIZE], wv_r[DSIZE]]),
        np.concatenate([wk_r[DSIZE + 1], wv_r[DSIZE + 1]]),
        np.concatenate([bk_r, bv_r])]))

    # pack the small weights into one bf16 blob (col map mirrors _build_program)
    blob16 = np.zeros((128, 2560), dtype=BF16)
    col = [0]

    def put(arr, width, rows=128):
        a = np.asarray(arr, dtype=np.float32).astype(BF16)
        blob16[:a.shape[0], col[0]:col[0] + width][:, :a.shape[1]] = a
        col[0] += width

    for pre, (wq_, bq_, wkv_, wkvp_, wo_, mlp_) in (
            ("f", (wq_f, bq_f, wkv_f, wkvp_f, wo_f, fw["mlp"])),
            ("r", (wq_r, bq_r, wkv_r_c, wkvp_r, wo_r, rv["mlp"]))):
        put(wq_, 128)
        put(np.tile(bq_, (128, 1)), 128)
        put(wkv_, 256)
        put(wkvp_, 256)
        put(wo_, 128)
        put(mlp_["w1"], 128)
        put(mlp_["w2"], 128)
        put(mlp_["w3"], 128)
    blob32 = np.stack([
        _f32(bo_f), _f32(fw["mlp"]["b1"]), _f32(fw["mlp"]["b2"]),
        _f32(fw["mlp"]["b3"]), _f32(bo_r), _f32(rv["mlp"]["b1"]),
        _f32(rv["mlp"]["b2"]), _f32(rv["mlp"]["b3"])], axis=1)

    shared = {
        "nfT16": _bf(node_features.T),
        "posones": _bf(np.stack([node_pos, np.ones(N, np.float32)])),
        "blob16": blob16,
        "blob32": _f32(blob32),
    }

    in_maps = []
    for c in range(NCORES):
        plo, phi = c * PD, (c + 1) * PD
        nlo, nhi = c * ND, (c + 1) * ND
        fw_flat = _gather_order(distance_structure[plo:phi])
        rv_flat = _gather_order(node_structure[nlo:nhi])
        uniq, inv = np.unique(rv_flat, return_inverse=True)
        nu = uniq.shape[0]
        upad = np.zeros(ND * K, dtype=np.int64)
        upad[:nu] = uniq
        raw_u = distance_features[upad]                       # [4096, 128]
        pos3 = np.stack([p0[upad], p1[upad],
                         np.ones(ND * K, np.float32)])
        m = dict(shared)
        m["dfT"] = _f32(distance_features[plo:phi].T)
        m["noiseT"] = _f32(noise_d[plo:phi].T)
        m["nfTs"] = _f32(node_features[nlo:nhi].T)
        m["noisenT"] = _f32(noise_n[nlo:nhi].T)
        m["fwidx"] = _wrap_idx16(fw_flat)
        m["rvidx"] = _wrap_idx16(inv.astype(np.int64))
        m["rawuT16"] = _bf(raw_u.T)
        m["pos3u"] = _bf(pos3)
        in_maps.append(m)
    return in_maps


_RUN_CACHE = {}


def _run(in_maps, trace=False, **kw):
    from concourse.bass_utils import run_bass_kernel_spmd

    nc = _build_program()
    return run_bass_kernel_spmd(nc, in_maps, core_ids=list(range(NCORES)),
                                trace=trace, **kw)


def kernel(**inputs):
    in_maps = _prepare(inputs)
    res = _run(in_maps)
    out_n = np.concatenate([res.results[c]["out_n"] for c in range(NCORES)])
    out_d = np.concatenate([res.results[c]["out_d"] for c in range(NCORES)])
    return out_n.astype(np.float32), out_d.astype(np.float32)
